# revision 5
# baseline (speedup 1.0000x reference)
"""Entmax-1.5 over rows of a (2048, 32000) fp32 tensor on 8 Trainium2 NeuronCores.

Per row, with raw-units threshold c (y = relu((x - c)/2)^2, sum y = 1):
  1. SWDGE cast-DMA loads x as fp16 tiles; DVE TT-max fold chain reduces
     each 128-row block to M[1000] strided chunk maxima (2x fp16 mode).
  2. 8x subrange top-8 (DVE max8) -> 64 candidates; warm Newton (3 iters,
     fp32, init = candidate mean) on the candidates -> c_w and
     sig_warm ~= full derivative (validated: slight underestimate of the
     derivative overshoots the from-below Newton step toward the root).
  3. relu pass in place (DVE tensor_scalar 4x fp16, no accum).
  4. f0 = sum (r/2)^2: ScalarE Square-accum chunks (-> PSUM trash) plus
     a couple of DVE stt-square-accum chunks (-> dead fold buffer).
  5. Newton on ScalarE (Identity/Relu with per-partition scale/bias APs):
     dc = max(0, (f0-1)*2/sig_warm), nh = -dc/2.
  6. out pass split: ScalarE Square(0.5 r + nh) -> fp16 bounce, and DVE
     shift+self-mult in place; DMA per half to fp16 DRAM output.

Warm candidates are solved in 2x units (v' = 2v, C = 2c) so the Newton
update needs no extra scaling ops.

Host: shard rows 8 ways, gather, cast fp16 -> fp32.
Validated vs float64 reference: rel err ~1.7e-3 (gate 2e-2).
"""

import numpy as np

import concourse.bass as bass
import concourse.bacc as bacc
import concourse.mybir as mybir
from concourse.tile import TileContext
from concourse.bass_utils import run_bass_kernel_spmd

f32 = mybir.dt.float32
f16 = mybir.dt.float16
Alu = mybir.AluOpType
Act = mybir.ActivationFunctionType
AxX = mybir.AxisListType.X

ROWS_TOTAL = 2048
V = 32000
N_CORES = 8
ROWS_PER_CORE = ROWS_TOTAL // N_CORES  # 256
P = 128
N_BLOCKS = ROWS_PER_CORE // P          # 2
TILE_W = 16000
NT = V // TILE_W                       # 2 tiles per block
HALF = TILE_W // 2                     # 8000
MW = 1000
NRANGE = 8
K = 8 * NRANGE                         # 64
WARM_ITERS = 3
F0_DVE_CHUNKS = 2                      # of 8 4000-wide chunks per block
OUT_SCALAR_HALVES = 2                  # of 4 8000-wide halves per block


class _Blk:
    pass


def build_kernel(nc: bass.Bass):
    x = nc.dram_tensor("x", [ROWS_PER_CORE, V], f32, kind="ExternalInput").ap()
    y = nc.dram_tensor("y", [ROWS_PER_CORE, V], f16, kind="ExternalOutput").ap()

    with TileContext(nc) as tc:
        with (
            tc.tile_pool(name="data", bufs=2 * NT) as dpool,
            tc.tile_pool(name="fold", bufs=2) as gpool,
            tc.tile_pool(name="ybuf", bufs=2) as ypool,
            tc.tile_pool(name="small", bufs=2) as spool,
            tc.tile_pool(name="psum", bufs=1, space="PSUM") as ppool,
        ):
            def sm(tag, cols=1, dt=f32):
                return spool.tile([P, cols], dt, tag=tag, name=tag)

            def new_block(b):
                s = _Blk()
                s.rows = slice(b * P, (b + 1) * P)
                s.xt = []
                return s

            def load(s, name):
                with nc.named_scope(f"load{name}"):
                    for t in range(NT):
                        xt = dpool.tile([P, TILE_W], f16, tag="xt", name="xt")
                        s.xt.append(xt)
                        cs = slice(t * TILE_W, (t + 1) * TILE_W)
                        nc.gpsimd.dma_start(out=xt, in_=x[s.rows, cs])

            def fold(s, name):
                with nc.named_scope(f"fold{name}"):
                    G = gpool.tile([P, HALF], f16, tag="G", name="G")
                    s.G = G
                    nc.vector.tensor_tensor(out=G, in0=s.xt[0][:, 0:HALF],
                                            in1=s.xt[0][:, HALF:TILE_W],
                                            op=Alu.max)
                    nc.vector.tensor_tensor(out=G, in0=G,
                                            in1=s.xt[1][:, 0:HALF], op=Alu.max)
                    nc.vector.tensor_tensor(out=G, in0=G,
                                            in1=s.xt[1][:, HALF:TILE_W],
                                            op=Alu.max)
                    w = HALF
                    while w > MW:
                        h = w // 2
                        nc.vector.tensor_tensor(out=G[:, 0:h], in0=G[:, 0:h],
                                                in1=G[:, h:w], op=Alu.max)
                        w = h

            def warm(s, name):
                with nc.named_scope(f"warm{name}"):
                    VK = sm("VK", K, f16)
                    W = MW // NRANGE
                    for i in range(NRANGE):
                        nc.vector.max(out=VK[:, 8 * i:8 * i + 8],
                                      in_=s.G[:, W * i:W * (i + 1)])
                    # 2x units: VKf = 2 v
                    VKf = sm("VKf", K)
                    nc.vector.tensor_scalar_mul(VKf, VK, 2.0)
                    vmax2 = sm("vmax2")
                    nc.vector.tensor_reduce(out=vmax2, in_=VKf, axis=AxX,
                                            op=Alu.max)
                    vsum = sm("vsum")
                    nc.vector.tensor_reduce(out=vsum, in_=VKf, axis=AxX,
                                            op=Alu.add)
                    C = sm("C")
                    nc.vector.tensor_scalar_mul(C, vsum, 1.0 / K)
                    z0 = sm("z0")
                    nc.vector.memset(z0, 0.0)
                    zb = z0.to_broadcast([P, K])
                    rV, rV2 = sm("rV", K), sm("rV2", K)
                    S, Q, rs, u = sm("S"), sm("Q"), sm("rs"), sm("u")
                    for _ in range(WARM_ITERS):
                        nc.vector.scalar_tensor_tensor(
                            out=rV, in0=VKf, scalar=C, in1=zb,
                            op0=Alu.subtract, op1=Alu.max, accum_out=S)
                        nc.vector.scalar_tensor_tensor(
                            out=rV2, in0=rV, scalar=1.0, in1=rV,
                            op0=Alu.mult, op1=Alu.mult, accum_out=Q)
                        nc.vector.reciprocal(rs, S)
                        # u = (Q - 16)/S = 2*dc2 ; C += u/2
                        nc.vector.scalar_tensor_tensor(
                            out=u, in0=Q, scalar=16.0, in1=rs,
                            op0=Alu.subtract, op1=Alu.mult)
                        nc.vector.scalar_tensor_tensor(
                            out=C, in0=u, scalar=0.5, in1=C,
                            op0=Alu.mult, op1=Alu.add)
                    cw2 = sm("cw2")
                    nc.vector.tensor_scalar(out=cw2, in0=vmax2, scalar1=2e-6,
                                            scalar2=C, op0=Alu.subtract,
                                            op1=Alu.min)
                    Sw = sm("Sw")
                    nc.vector.scalar_tensor_tensor(
                        out=rV, in0=VKf, scalar=cw2, in1=zb,
                        op0=Alu.subtract, op1=Alu.max, accum_out=Sw)
                    cw = sm("cw")
                    nc.vector.tensor_scalar_mul(cw, cw2, 0.5)
                    rsig = sm("rsig")       # 1/(2 sigw)
                    nc.vector.reciprocal(rsig, Sw)
                    nrsig = sm("nrsig")
                    nc.vector.tensor_scalar_mul(nrsig, rsig, -1.0)
                    s.cw, s.rsig, s.nrsig = cw, rsig, nrsig

            def iter_block(s, name):
                with nc.named_scope(f"iter{name}"):
                    f0c = sm("f0c", 8)
                    # relu per half (DVE 4x), then f0 per 4000-chunk
                    chunk = 0
                    dve_chunks = []
                    for t in range(NT):
                        for h in range(2):
                            hs = slice(h * HALF, (h + 1) * HALF)
                            nc.vector.tensor_scalar(
                                out=s.xt[t][:, hs], in0=s.xt[t][:, hs],
                                scalar1=s.cw, scalar2=0.0,
                                op0=Alu.subtract, op1=Alu.max)
                            for q in range(2):
                                lo = h * HALF + q * 4000
                                csl = slice(lo, lo + 4000)
                                if chunk >= 8 - F0_DVE_CHUNKS:
                                    dve_chunks.append((t, csl, chunk))
                                else:
                                    ps = ppool.tile([P, 4000], f32, tag="ps",
                                                    name="ps")
                                    nc.scalar.activation(
                                        out=ps, in_=s.xt[t][:, csl],
                                        func=Act.Square, scale=0.5,
                                        accum_out=f0c[:, chunk:chunk + 1])
                                chunk += 1
                    for (t, csl, ci) in dve_chunks:
                        # (r * 0.25) * r accumulated = sum (r/2)^2
                        gdst = s.G[:, 0:4000] if ci % 2 == 0 else s.G[:, 4000:HALF]
                        nc.vector.scalar_tensor_tensor(
                            out=gdst, in0=s.xt[t][:, csl], scalar=0.25,
                            in1=s.xt[t][:, csl], op0=Alu.mult, op1=Alu.mult,
                            accum_out=f0c[:, ci:ci + 1])
                    f0 = sm("f0")
                    nc.vector.tensor_reduce(out=f0, in_=f0c, axis=AxX,
                                            op=Alu.add)
                    # newton smalls on ScalarE
                    dc0, dc, nh = sm("dc0"), sm("dc"), sm("nh")
                    nc.scalar.activation(out=dc0, in_=f0, func=Act.Identity,
                                         scale=s.rsig, bias=s.nrsig)
                    nc.scalar.activation(out=dc, in_=dc0, func=Act.Relu,
                                         scale=4.0)
                    nc.scalar.activation(out=nh, in_=dc, func=Act.Identity,
                                         scale=-0.5)
                    s.dc, s.nh = dc, nh

            def out_block(s, name):
                with nc.named_scope(f"out{name}"):
                    half = 0
                    for t in range(NT):
                        for h in range(2):
                            hs = slice(h * HALF, (h + 1) * HALF)
                            glo = t * TILE_W + h * HALF
                            gsl = slice(glo, glo + HALF)
                            if half < OUT_SCALAR_HALVES:
                                yb = ypool.tile([P, HALF], f16, tag="yb",
                                                name="yb")
                                nc.scalar.activation(out=yb,
                                                     in_=s.xt[t][:, hs],
                                                     func=Act.Square,
                                                     scale=0.5, bias=s.nh)
                                nc.sync.dma_start(out=y[s.rows, gsl], in_=yb)
                            else:
                                nc.vector.tensor_scalar(
                                    out=s.xt[t][:, hs], in0=s.xt[t][:, hs],
                                    scalar1=s.dc, scalar2=0.5,
                                    op0=Alu.subtract, op1=Alu.mult)
                                nc.vector.tensor_tensor(
                                    out=s.xt[t][:, hs], in0=s.xt[t][:, hs],
                                    in1=s.xt[t][:, hs], op=Alu.mult)
                                nc.sync.dma_start(out=y[s.rows, gsl],
                                                  in_=s.xt[t][:, hs])
                            half += 1

            A, B = new_block(0), new_block(1)
            load(A, "A")
            fold(A, "A")
            load(B, "B")
            warm(A, "A")
            iter_block(A, "A")
            fold(B, "B")
            out_block(A, "A")
            warm(B, "B")
            iter_block(B, "B")
            out_block(B, "B")
    return nc


_COMPILED = {}


def _get_nc():
    if "nc" not in _COMPILED:
        nc = bacc.Bacc("TRN2", target_bir_lowering=False, debug=False,
                       num_devices=N_CORES)
        build_kernel(nc)
        nc.compile()
        _COMPILED["nc"] = nc
    return _COMPILED["nc"]


def kernel(X: np.ndarray) -> np.ndarray:
    assert X.shape == (ROWS_TOTAL, V) and X.dtype == np.float32, (X.shape, X.dtype)
    nc = _get_nc()
    in_maps = [
        {"x": np.ascontiguousarray(X[i * ROWS_PER_CORE:(i + 1) * ROWS_PER_CORE])}
        for i in range(N_CORES)
    ]
    res = run_bass_kernel_spmd(nc, in_maps, core_ids=list(range(N_CORES)))
    return np.concatenate(
        [r["y"].astype(np.float32) for r in res.results], axis=0)


# revision 7
# speedup vs baseline: 1.0879x; 1.0879x over previous
"""Entmax-1.5 over rows of a (2048, 32000) fp32 tensor on 8 Trainium2 NeuronCores.

Per row, with raw-units threshold c (y = relu((x - c)/2)^2, sum y = 1):
  1. SWDGE cast-DMA loads x as fp16 tiles (widths 16000/8000/4000/4000 per
     128-row block, so the TT-max fold chain finishes right after the last
     DMA); fold gives M[1000] strided chunk maxima per block.
  2. 8x subrange top-8 (DVE max8) -> 64 candidates; 3 warm Newton iters
     (fp32, init = candidate mean) -> c_w, with 1/sig taken from the last
     iteration (validated numerically).
  3. relu pass in place (DVE tensor_scalar 4x fp16, no accum).
  4. f0 = sum (r/2)^2 via ScalarE Square-accum chunks (PSUM trash) plus
     DVE stt-square-accum chunks (dead fold buffer as trash).
  5. Newton on ScalarE: dc = max(0, (f0-1)*2/sig), nh = -dc/2.
  6. out pass split between ScalarE Square(0.5 r + nh) -> fp16 bounce and
     DVE shift+self-mult in place; DMA units to fp16 DRAM output.

Emission order is chosen for the in-order per-engine programs so block B's
fold/warm and block A's iter/out interleave without blocking each other.

Host: shard rows 8 ways, gather, cast fp16 -> fp32.
Validated vs float64 reference: rel err ~1.7e-3 (gate 2e-2).
"""

import os
import numpy as np

import concourse.bass as bass
import concourse.bacc as bacc
import concourse.mybir as mybir
from concourse.tile import TileContext
from concourse.bass_utils import run_bass_kernel_spmd

f32 = mybir.dt.float32
f16 = mybir.dt.float16
Alu = mybir.AluOpType
Act = mybir.ActivationFunctionType
AxX = mybir.AxisListType.X

ROWS_TOTAL = 2048
V = 32000
N_CORES = 8
ROWS_PER_CORE = ROWS_TOTAL // N_CORES  # 256
P = 128
N_BLOCKS = 2
TILE_WS = [16000, 8000, 4000, 4000]    # per-block tile widths (sum 32000)
MW = 1000
NRANGE = 8
K = 8 * NRANGE
WARM_ITERS = 3
# out units per block: (tile, lo, width) - 3x8000 + 2x4000
OUT_UNITS = [(0, 0, 8000), (0, 8000, 8000), (1, 0, 8000),
             (2, 0, 4000), (3, 0, 4000)]
F0_DVE = int(os.environ.get("F0_DVE", "2"))          # of 8 4000-chunks
OUT_S = int(os.environ.get("OUT_S", "2"))            # of 5 units on ScalarE


class _Blk:
    pass


def build_kernel(nc: bass.Bass):
    x = nc.dram_tensor("x", [ROWS_PER_CORE, V], f32, kind="ExternalInput").ap()
    y = nc.dram_tensor("y", [ROWS_PER_CORE, V], f16, kind="ExternalOutput").ap()

    with TileContext(nc) as tc:
        with (
            tc.tile_pool(name="data", bufs=2) as dpool,
            tc.tile_pool(name="fold", bufs=2) as gpool,
            tc.tile_pool(name="ybuf", bufs=2) as ypool,
            tc.tile_pool(name="small", bufs=2) as spool,
            tc.tile_pool(name="psum", bufs=1, space="PSUM") as ppool,
        ):
            def sm(tag, cols=1, dt=f32):
                return spool.tile([P, cols], dt, tag=tag, name=tag)

            z0 = spool.tile([P, 1], f32, tag="z0", name="z0", bufs=1)
            nc.vector.memset(z0, 0.0)
            zb = z0.to_broadcast([P, K])

            def new_block(b):
                s = _Blk()
                s.rows = slice(b * P, (b + 1) * P)
                s.xt = []
                return s

            def load(s, name):
                with nc.named_scope(f"load{name}"):
                    off = 0
                    for w in TILE_WS:
                        xt = dpool.tile([P, w], f16, tag=f"xt{w}_{len(s.xt)}",
                                        name="xt")
                        s.xt.append(xt)
                        nc.gpsimd.dma_start(out=xt,
                                            in_=x[s.rows, off:off + w])
                        off += w

            def fold(s, name, emit):
                """Emit fold TT ops; `emit` selects chain stages so callers
                can interleave other Vector work between data arrivals."""
                with nc.named_scope(f"fold{name}"):
                    if emit == "head":
                        G = gpool.tile([P, 8000], f16, tag="G", name="G")
                        s.G = G
                        nc.vector.tensor_tensor(
                            out=G, in0=s.xt[0][:, 0:8000],
                            in1=s.xt[0][:, 8000:16000], op=Alu.max)
                        nc.vector.tensor_tensor(out=G, in0=G, in1=s.xt[1],
                                                op=Alu.max)
                        G4 = G[:, 0:4000]
                        nc.vector.tensor_tensor(out=G4, in0=G4,
                                                in1=G[:, 4000:8000],
                                                op=Alu.max)
                        nc.vector.tensor_tensor(out=G4, in0=G4, in1=s.xt[2],
                                                op=Alu.max)
                    else:
                        G = s.G
                        G4 = G[:, 0:4000]
                        nc.vector.tensor_tensor(out=G4, in0=G4, in1=s.xt[3],
                                                op=Alu.max)
                        nc.vector.tensor_tensor(out=G[:, 0:2000],
                                                in0=G[:, 0:2000],
                                                in1=G[:, 2000:4000],
                                                op=Alu.max)
                        nc.vector.tensor_tensor(out=G[:, 0:1000],
                                                in0=G[:, 0:1000],
                                                in1=G[:, 1000:2000],
                                                op=Alu.max)

            def warm(s, name):
                with nc.named_scope(f"warm{name}"):
                    VK = sm("VK", K, f16)
                    W = MW // NRANGE
                    for i in range(NRANGE):
                        nc.vector.max(out=VK[:, 8 * i:8 * i + 8],
                                      in_=s.G[:, W * i:W * (i + 1)])
                    VKf = sm("VKf", K)
                    nc.vector.tensor_copy(VKf, VK)
                    vsum = sm("vsum")
                    nc.vector.tensor_reduce(out=vsum, in_=VKf, axis=AxX,
                                            op=Alu.add)
                    C = sm("C")
                    nc.vector.tensor_scalar_mul(C, vsum, 1.0 / K)
                    rV, rV2 = sm("rV", K), sm("rV2", K)
                    S, Q, rs, u = sm("S"), sm("Q"), sm("rs"), sm("u")
                    for _ in range(WARM_ITERS):
                        nc.vector.scalar_tensor_tensor(
                            out=rV, in0=VKf, scalar=C, in1=zb,
                            op0=Alu.subtract, op1=Alu.max, accum_out=S)
                        nc.vector.scalar_tensor_tensor(
                            out=rV2, in0=rV, scalar=1.0, in1=rV,
                            op0=Alu.mult, op1=Alu.mult, accum_out=Q)
                        nc.vector.reciprocal(rs, S)
                        nc.vector.scalar_tensor_tensor(
                            out=u, in0=Q, scalar=4.0, in1=rs,
                            op0=Alu.subtract, op1=Alu.mult)
                        nc.vector.scalar_tensor_tensor(
                            out=C, in0=u, scalar=0.5, in1=C,
                            op0=Alu.mult, op1=Alu.add)
                    nrsig = sm("nrsig")
                    nc.vector.tensor_scalar_mul(nrsig, rs, -1.0)
                    s.cw, s.rsig, s.nrsig = C, rs, nrsig

            def relu_and_f0(s, name):
                """DVE relu per unit; ScalarE Square-accum chunks for the
                first 8-F0_DVE 4000-chunks; DVE stt chunks for the rest."""
                with nc.named_scope(f"iter{name}"):
                    f0c = sm("f0c", 8)
                    s.f0c = f0c
                    chunk = 0
                    s.dve_chunks = []
                    for (t, lo, w) in OUT_UNITS:
                        sl = slice(lo, lo + w)
                        nc.vector.tensor_scalar(
                            out=s.xt[t][:, sl], in0=s.xt[t][:, sl],
                            scalar1=s.cw, scalar2=0.0,
                            op0=Alu.subtract, op1=Alu.max)
                        for q in range(w // 4000):
                            csl = slice(lo + q * 4000, lo + (q + 1) * 4000)
                            if chunk >= 8 - F0_DVE:
                                s.dve_chunks.append((t, csl, chunk))
                            else:
                                ps = ppool.tile([P, 4000], f32, tag="ps",
                                                name="ps")
                                nc.scalar.activation(
                                    out=ps, in_=s.xt[t][:, csl],
                                    func=Act.Square, scale=0.5,
                                    accum_out=f0c[:, chunk:chunk + 1])
                            chunk += 1
                    for (t, csl, ci) in s.dve_chunks:
                        gdst = s.G[:, 0:4000] if ci % 2 == 0 \
                            else s.G[:, 4000:8000]
                        nc.vector.scalar_tensor_tensor(
                            out=gdst, in0=s.xt[t][:, csl], scalar=0.25,
                            in1=s.xt[t][:, csl], op0=Alu.mult, op1=Alu.mult,
                            accum_out=f0c[:, ci:ci + 1])

            def newton(s, name):
                with nc.named_scope(f"newt{name}"):
                    f0 = sm("f0")
                    nc.vector.tensor_reduce(out=f0, in_=s.f0c, axis=AxX,
                                            op=Alu.add)
                    dc0, dc, nh = sm("dc0"), sm("dc"), sm("nh")
                    nc.scalar.activation(out=dc0, in_=f0, func=Act.Identity,
                                         scale=s.rsig, bias=s.nrsig)
                    nc.scalar.activation(out=dc, in_=dc0, func=Act.Relu,
                                         scale=2.0)
                    nc.scalar.activation(out=nh, in_=dc, func=Act.Identity,
                                         scale=-0.5)
                    s.dc, s.nh = dc, nh

            def out_scalar(s, name):
                with nc.named_scope(f"out{name}"):
                    for (t, lo, w) in OUT_UNITS[:OUT_S]:
                        sl = slice(lo, lo + w)
                        glo = sum(TILE_WS[:t]) + lo
                        yb = ypool.tile([P, w], f16, tag=f"yb{w}", name="yb")
                        nc.scalar.activation(out=yb, in_=s.xt[t][:, sl],
                                             func=Act.Square, scale=0.5,
                                             bias=s.nh)
                        nc.sync.dma_start(out=y[s.rows, glo:glo + w], in_=yb)

            def out_dve(s, name):
                with nc.named_scope(f"out{name}"):
                    for (t, lo, w) in OUT_UNITS[OUT_S:]:
                        sl = slice(lo, lo + w)
                        glo = sum(TILE_WS[:t]) + lo
                        nc.vector.tensor_scalar(
                            out=s.xt[t][:, sl], in0=s.xt[t][:, sl],
                            scalar1=s.dc, scalar2=0.5,
                            op0=Alu.subtract, op1=Alu.mult)
                        nc.vector.tensor_tensor(
                            out=s.xt[t][:, sl], in0=s.xt[t][:, sl],
                            in1=s.xt[t][:, sl], op=Alu.mult)
                        nc.sync.dma_start(out=y[s.rows, glo:glo + w],
                                          in_=s.xt[t][:, sl])

            A, B = new_block(0), new_block(1)
            load(A, "A")
            fold(A, "A", "head")
            load(B, "B")
            fold(A, "A", "tail")
            warm(A, "A")
            relu_and_f0(A, "A")
            fold(B, "B", "head")
            newton(A, "A")          # f0 reduce (V) + newton smalls (S)
            out_scalar(A, "A")      # Scalar program continues with A out
            fold(B, "B", "tail")
            warm(B, "B")
            out_dve(A, "A")
            relu_and_f0(B, "B")
            newton(B, "B")
            out_scalar(B, "B")
            out_dve(B, "B")
    return nc


_COMPILED = {}


def _get_nc():
    if "nc" not in _COMPILED:
        nc = bacc.Bacc("TRN2", target_bir_lowering=False, debug=False,
                       num_devices=N_CORES)
        build_kernel(nc)
        nc.compile()
        _COMPILED["nc"] = nc
    return _COMPILED["nc"]


def kernel(X: np.ndarray) -> np.ndarray:
    assert X.shape == (ROWS_TOTAL, V) and X.dtype == np.float32, (X.shape, X.dtype)
    nc = _get_nc()
    in_maps = [
        {"x": np.ascontiguousarray(X[i * ROWS_PER_CORE:(i + 1) * ROWS_PER_CORE])}
        for i in range(N_CORES)
    ]
    res = run_bass_kernel_spmd(nc, in_maps, core_ids=list(range(N_CORES)))
    return np.concatenate(
        [r["y"].astype(np.float32) for r in res.results], axis=0)


# revision 12
# speedup vs baseline: 1.2318x; 1.1322x over previous
"""Entmax-1.5 over rows of a (2048, 32000) fp32 tensor on 8 Trainium2 NeuronCores.

Per row, with raw-units threshold c (y = relu((x - c)/2)^2, sum y = 1):
  1. SWDGE cast-DMA loads x as fp16 tiles (widths 16000/8000/4000/4000 per
     128-row block, so the TT-max fold chain finishes right after the last
     DMA); fold gives M[1000] strided chunk maxima per block.
  2. 8x subrange top-8 (DVE max8) -> 64 candidates; 3 warm Newton iters
     (fp32, init = candidate mean) -> c_w, with 1/sig taken from the last
     iteration (validated numerically).
  3. relu pass in place (DVE tensor_scalar 4x fp16, no accum).
  4. f0 = sum (r/2)^2 via ScalarE Square-accum chunks (PSUM trash) plus
     DVE stt-square-accum chunks (dead fold buffer as trash).
  5. Newton on ScalarE: dc = max(0, (f0-1)*2/sig), nh = -dc/2.
  6. out pass split between ScalarE Square(0.5 r + nh) -> fp16 bounce and
     DVE shift+self-mult in place; DMA units to fp16 DRAM output.

Emission order is chosen for the in-order per-engine programs so block B's
fold/warm and block A's iter/out interleave without blocking each other.

Host: shard rows 8 ways, gather, cast fp16 -> fp32.
Validated vs float64 reference: rel err ~1.7e-3 (gate 2e-2).
"""

import os
import numpy as np

import concourse.bass as bass
import concourse.bacc as bacc
import concourse.mybir as mybir
from concourse.tile import TileContext
from concourse.bass_utils import run_bass_kernel_spmd

f32 = mybir.dt.float32
f16 = mybir.dt.float16
Alu = mybir.AluOpType
Act = mybir.ActivationFunctionType
AxX = mybir.AxisListType.X

ROWS_TOTAL = 2048
V = 32000
N_CORES = 8
ROWS_PER_CORE = ROWS_TOTAL // N_CORES  # 256
P = 128
N_BLOCKS = 2
TILE_WS = [16000, 8000, 4000, 4000]    # per-block tile widths (sum 32000)
MW = 1000
NRANGE = 8
K = 8 * NRANGE
WARM_ITERS = 3
# out units per block: (tile, lo, width) - 3x8000 + 2x4000
OUT_UNITS = [(0, 0, 8000), (0, 8000, 8000), (1, 0, 8000),
             (2, 0, 4000), (3, 0, 4000)]
# f0 computed per unit: units listed here go to DVE (stt square-accum into
# the dead fold buffer), the rest to ScalarE Square-accum (8000-wide units
# bounce via ybuf trash, 4000-wide via PSUM trash).
F0_V_UNITS = tuple(int(c) for c in os.environ.get("F0V", "24"))
OUT_S_A = int(os.environ.get("OUT_S_A", "3"))        # A units on ScalarE
OUT_S_B = int(os.environ.get("OUT_S_B", "2"))        # B units on ScalarE


class _Blk:
    pass


def build_kernel(nc: bass.Bass):
    x = nc.dram_tensor("x", [ROWS_PER_CORE, V], f32, kind="ExternalInput").ap()
    y = nc.dram_tensor("y", [ROWS_PER_CORE, V], f16, kind="ExternalOutput").ap()

    with TileContext(nc) as tc:
        with (
            tc.tile_pool(name="data", bufs=2) as dpool,
            tc.tile_pool(name="fold", bufs=2) as gpool,
            tc.tile_pool(name="ybuf", bufs=2) as ypool,
            tc.tile_pool(name="small", bufs=2) as spool,
            tc.tile_pool(name="psum", bufs=1, space="PSUM") as ppool,
        ):
            def sm(tag, cols=1, dt=f32):
                return spool.tile([P, cols], dt, tag=tag, name=tag)

            z0 = spool.tile([P, 1], f32, tag="z0", name="z0", bufs=1)
            nc.vector.memset(z0, 0.0)
            zb = z0.to_broadcast([P, K])

            def new_block(b):
                s = _Blk()
                s.rows = slice(b * P, (b + 1) * P)
                s.xt = []
                return s

            def load(s, name):
                with nc.named_scope(f"load{name}"):
                    off = 0
                    for w in TILE_WS:
                        xt = dpool.tile([P, w], f16, tag=f"xt{w}_{len(s.xt)}",
                                        name="xt")
                        s.xt.append(xt)
                        nc.gpsimd.dma_start(out=xt,
                                            in_=x[s.rows, off:off + w])
                        off += w

            def fold(s, name, emit):
                """Emit fold TT ops; `emit` selects chain stages so callers
                can interleave other Vector work between data arrivals."""
                with nc.named_scope(f"fold{name}"):
                    if emit == "head":
                        G = gpool.tile([P, 8000], f16, tag="G", name="G")
                        s.G = G
                        nc.vector.tensor_tensor(
                            out=G, in0=s.xt[0][:, 0:8000],
                            in1=s.xt[0][:, 8000:16000], op=Alu.max)
                        nc.vector.tensor_tensor(out=G, in0=G, in1=s.xt[1],
                                                op=Alu.max)
                        G4 = G[:, 0:4000]
                        nc.vector.tensor_tensor(out=G4, in0=G4,
                                                in1=G[:, 4000:8000],
                                                op=Alu.max)
                        nc.vector.tensor_tensor(out=G4, in0=G4, in1=s.xt[2],
                                                op=Alu.max)
                    else:
                        G = s.G
                        G4 = G[:, 0:4000]
                        nc.vector.tensor_tensor(out=G4, in0=G4, in1=s.xt[3],
                                                op=Alu.max)
                        nc.vector.tensor_tensor(out=G[:, 0:2000],
                                                in0=G[:, 0:2000],
                                                in1=G[:, 2000:4000],
                                                op=Alu.max)
                        nc.vector.tensor_tensor(out=G[:, 0:1000],
                                                in0=G[:, 0:1000],
                                                in1=G[:, 1000:2000],
                                                op=Alu.max)

            def warm(s, name):
                with nc.named_scope(f"warm{name}"):
                    VK = sm("VK", K, f16)
                    W = MW // NRANGE
                    for i in range(NRANGE):
                        nc.vector.max(out=VK[:, 8 * i:8 * i + 8],
                                      in_=s.G[:, W * i:W * (i + 1)])
                    VKf = sm("VKf", K)
                    nc.vector.tensor_copy(VKf, VK)
                    vsum = sm("vsum")
                    nc.vector.tensor_reduce(out=vsum, in_=VKf, axis=AxX,
                                            op=Alu.add)
                    C = sm("C")
                    nc.vector.tensor_scalar_mul(C, vsum, 1.0 / K)
                    rV, rV2 = sm("rV", K), sm("rV2", K)
                    S, Q, rs, u = sm("S"), sm("Q"), sm("rs"), sm("u")
                    for _ in range(WARM_ITERS):
                        nc.vector.scalar_tensor_tensor(
                            out=rV, in0=VKf, scalar=C, in1=zb,
                            op0=Alu.subtract, op1=Alu.max, accum_out=S)
                        nc.vector.scalar_tensor_tensor(
                            out=rV2, in0=rV, scalar=1.0, in1=rV,
                            op0=Alu.mult, op1=Alu.mult, accum_out=Q)
                        nc.vector.reciprocal(rs, S)
                        nc.vector.scalar_tensor_tensor(
                            out=u, in0=Q, scalar=4.0, in1=rs,
                            op0=Alu.subtract, op1=Alu.mult)
                        nc.vector.scalar_tensor_tensor(
                            out=C, in0=u, scalar=0.5, in1=C,
                            op0=Alu.mult, op1=Alu.add)
                    nrsig = sm("nrsig")
                    nc.vector.tensor_scalar_mul(nrsig, rs, -1.0)
                    s.cw, s.rsig, s.nrsig = C, rs, nrsig

            def relu_and_f0(s, name):
                """DVE relu per unit; f0 accum per unit on ScalarE (Square)
                or DVE (stt square), unit assignment via F0_V_UNITS."""
                with nc.named_scope(f"iter{name}"):
                    nu = len(OUT_UNITS)
                    f0c = sm("f0c", nu)
                    s.f0c = f0c
                    for ui, (t, lo, w) in enumerate(OUT_UNITS):
                        sl = slice(lo, lo + w)
                        nc.vector.tensor_scalar(
                            out=s.xt[t][:, sl], in0=s.xt[t][:, sl],
                            scalar1=s.cw, scalar2=0.0,
                            op0=Alu.subtract, op1=Alu.max)
                        if ui in F0_V_UNITS:
                            continue
                        if w == 8000:
                            tr = ypool.tile([P, w], f16, tag=f"yb{w}",
                                            name="yb")
                        else:
                            tr = ppool.tile([P, w], f32, tag="ps", name="ps")
                        nc.scalar.activation(
                            out=tr, in_=s.xt[t][:, sl],
                            func=Act.Square, scale=0.5,
                            accum_out=f0c[:, ui:ui + 1])
                    for ui in F0_V_UNITS:
                        t, lo, w = OUT_UNITS[ui]
                        sl = slice(lo, lo + w)
                        gdst = s.G[:, 0:w]
                        nc.vector.scalar_tensor_tensor(
                            out=gdst, in0=s.xt[t][:, sl], scalar=0.25,
                            in1=s.xt[t][:, sl], op0=Alu.mult, op1=Alu.mult,
                            accum_out=f0c[:, ui:ui + 1])

            def newton(s, name):
                with nc.named_scope(f"newt{name}"):
                    f0 = sm("f0")
                    nc.vector.tensor_reduce(out=f0, in_=s.f0c, axis=AxX,
                                            op=Alu.add)
                    dc0, dc, nh = sm("dc0"), sm("dc"), sm("nh")
                    nc.scalar.activation(out=dc0, in_=f0, func=Act.Identity,
                                         scale=s.rsig, bias=s.nrsig)
                    nc.scalar.activation(out=dc, in_=dc0, func=Act.Relu,
                                         scale=2.0)
                    nc.scalar.activation(out=nh, in_=dc, func=Act.Identity,
                                         scale=-0.5)
                    s.dc, s.nh = dc, nh

            def out_scalar(s, name, n_s):
                with nc.named_scope(f"out{name}"):
                    for (t, lo, w) in OUT_UNITS[:n_s]:
                        sl = slice(lo, lo + w)
                        glo = sum(TILE_WS[:t]) + lo
                        yb = ypool.tile([P, w], f16, tag=f"yb{w}", name="yb")
                        nc.scalar.activation(out=yb, in_=s.xt[t][:, sl],
                                             func=Act.Square, scale=0.5,
                                             bias=s.nh)
                        nc.sync.dma_start(out=y[s.rows, glo:glo + w], in_=yb)

            def out_dve(s, name, n_s):
                with nc.named_scope(f"out{name}"):
                    for (t, lo, w) in OUT_UNITS[n_s:]:
                        sl = slice(lo, lo + w)
                        glo = sum(TILE_WS[:t]) + lo
                        nc.vector.tensor_scalar(
                            out=s.xt[t][:, sl], in0=s.xt[t][:, sl],
                            scalar1=s.dc, scalar2=0.5,
                            op0=Alu.subtract, op1=Alu.mult)
                        nc.vector.tensor_tensor(
                            out=s.xt[t][:, sl], in0=s.xt[t][:, sl],
                            in1=s.xt[t][:, sl], op=Alu.mult)
                        nc.sync.dma_start(out=y[s.rows, glo:glo + w],
                                          in_=s.xt[t][:, sl])

            A, B = new_block(0), new_block(1)
            load(A, "A")
            fold(A, "A", "head")
            load(B, "B")
            fold(A, "A", "tail")
            warm(A, "A")
            relu_and_f0(A, "A")
            fold(B, "B", "head")
            newton(A, "A")          # f0 reduce (V) + newton smalls (S)
            out_scalar(A, "A", OUT_S_A)
            fold(B, "B", "tail")
            warm(B, "B")
            out_dve(A, "A", OUT_S_A)
            relu_and_f0(B, "B")
            newton(B, "B")
            out_scalar(B, "B", OUT_S_B)
            out_dve(B, "B", OUT_S_B)
    return nc


_COMPILED = {}


def _get_nc():
    if "nc" not in _COMPILED:
        nc = bacc.Bacc("TRN2", target_bir_lowering=False, debug=False,
                       num_devices=N_CORES)
        build_kernel(nc)
        nc.compile()
        _COMPILED["nc"] = nc
    return _COMPILED["nc"]


def kernel(X: np.ndarray) -> np.ndarray:
    assert X.shape == (ROWS_TOTAL, V) and X.dtype == np.float32, (X.shape, X.dtype)
    nc = _get_nc()
    in_maps = [
        {"x": np.ascontiguousarray(X[i * ROWS_PER_CORE:(i + 1) * ROWS_PER_CORE])}
        for i in range(N_CORES)
    ]
    res = run_bass_kernel_spmd(nc, in_maps, core_ids=list(range(N_CORES)))
    return np.concatenate(
        [r["y"].astype(np.float32) for r in res.results], axis=0)
